# revision 1
# baseline (speedup 1.0000x reference)
"""Trainium2 Bass kernel for 2-layer GCN (N=50000, E=600000, 128->512->128).

Strategy (8 NeuronCores, graph/data parallel over destination nodes):
  - Host: symmetric-normalization is separable (norm = dinv[src]*dinv[dst]),
    so the gather table rows are pre-scaled by dinv[src] and the aggregate is
    post-scaled by dinv[dst]. Nodes are packed into 8*49 windows of <=128
    destination nodes, balancing per-window edge counts so one SPMD program
    (fixed shapes) serves all cores. Edge slots are split into two source
    ranges (A: table rows [0,32768), B: rows [17234,50002)) so gather indices
    fit int16, padded per window to NA*128 / NB*128 slots.
  - Device, per window: dma_gather fp16 source rows (256B rows, two windows
    per gather, single_packet=False) -> build all of a window-range's one-hot
    S matrices with ONE wide DVE is_equal against a broadcast iota (batched to
    amortize per-op overhead) -> PE matmuls accumulate the aggregation in
    PSUM (operands swapped per layer so each consumer gets its layout without
    transposes). PSUM drains ride the otherwise-idle ACT engine. Layer 1
    continues on-chip: agg -> @W1^T -> +b1,relu -> @W2^T -> z (so only the
    128-wide z crosses HBM between layers). Layer 2 finishes:
    agg*dinv[dst] + b2 -> relu -> out.
  - Host between launches: reshuffles z shards into the layer-2 gather table
    (scaled by dinv), then un-permutes the final output.
"""

import heapq
import numpy as np

import concourse.bacc as bacc
import concourse.mybir as mybir
import concourse.tile as tile
from concourse.bass_utils import run_bass_kernel_spmd

# problem constants (hardcoded per contract)
N = 50000
E = 600000
F = 128          # in/out feature dim
H = 512          # hidden dim
P = 128
NCORES = 8
WPC = 49                  # windows per core
BINS = NCORES * WPC       # 392
ROWS_PER_CORE = WPC * P   # 6272 output rows per core (>= 6250 real)
TBL_ROWS = N + 2          # zero row at 0 and N+1
A_MAX_SRC = 32766         # srcs <= this go to range A (idx = src+1 <= 32767)
B_OFF = 17234             # range B table view starts at this row
B_PAD_IDX = 32767         # row N+1 (zero) relative to B view
SENTINEL = 300.0          # dstloc value that never matches iota 0..127
GW = 2                    # windows per gather group

last_run_info = {}


# ---------------------------------------------------------------- host planner
def _pack_bins(a_tot, b_tot, cap_a, cap_b):
    """Greedy balanced packing of nodes into BINS bins (<=P nodes, slot caps).
    Returns per-node bin id, or None if packing failed."""
    order = np.argsort(-(a_tot * 3 + b_tot))  # heaviest first
    bin_of = np.full(N, -1, np.int32)
    heap = [(0, 0, 0, b) for b in range(BINS)]  # (aload, bload, count, bin)
    heapq.heapify(heap)
    for n in order:
        a, b = int(a_tot[n]), int(b_tot[n])
        tried = []
        placed = False
        while heap:
            al, bl, cnt, bid = heapq.heappop(heap)
            if cnt >= P:
                continue  # bin full: drop permanently
            if al + a <= cap_a and bl + b <= cap_b:
                bin_of[n] = bid
                heapq.heappush(heap, (al + a, bl + b, cnt + 1, bid))
                placed = True
                break
            tried.append((al, bl, cnt, bid))
            if len(tried) > 256:
                break
        for t in tried:
            heapq.heappush(heap, t)
        if not placed:
            return None
    return bin_of


def build_plan(edge_index):
    src = np.asarray(edge_index[0], dtype=np.int64).astype(np.int32)
    dst = np.asarray(edge_index[1], dtype=np.int64).astype(np.int32)

    deg = np.bincount(dst, minlength=N).astype(np.int64) + 1  # + self loop
    dinv = (1.0 / np.sqrt(deg)).astype(np.float32)

    is_a = src <= A_MAX_SRC
    a_cnt = np.bincount(dst[is_a], minlength=N)
    b_cnt = np.bincount(dst[~is_a], minlength=N)
    self_a = np.arange(N) <= A_MAX_SRC
    a_tot = a_cnt + self_a
    b_tot = b_cnt + (~self_a)

    for na, nb in ((9, 5), (9, 6), (10, 6), (10, 8), (12, 10)):
        bin_of = _pack_bins(a_tot, b_tot, na * P, nb * P)
        if bin_of is not None:
            NA, NB = na, nb
            break
    else:
        raise RuntimeError("bin packing failed")

    # per-bin node lists / positions
    node_core = bin_of // WPC
    node_win = bin_of % WPC
    node_pos = np.zeros(N, np.int32)
    fill = np.zeros(BINS, np.int32)
    for n in range(N):
        b = bin_of[n]
        node_pos[n] = fill[b]
        fill[b] += 1

    # CSR of incoming edges per node (edges only; self loop added below)
    order = np.argsort(dst, kind="stable")
    src_sorted = src[order]
    starts = np.zeros(N + 1, np.int64)
    np.cumsum(np.bincount(dst, minlength=N), out=starts[1:])

    slots_a = NA * P
    slots_b = NB * P
    idxA = np.zeros((NCORES, WPC, slots_a), np.int16)
    idxB = np.full((NCORES, WPC, slots_b), B_PAD_IDX, np.int16)
    dlA = np.full((NCORES, WPC, slots_a), SENTINEL, np.float32)
    dlB = np.full((NCORES, WPC, slots_b), SENTINEL, np.float32)
    fa = np.zeros((NCORES, WPC), np.int32)
    fb = np.zeros((NCORES, WPC), np.int32)
    dinvw = np.zeros((NCORES, WPC, P), np.float32)

    for n in range(N):
        c, w, p = node_core[n], node_win[n], node_pos[n]
        dinvw[c, w, p] = dinv[n]
        es = src_sorted[starts[n]:starts[n + 1]]
        ea = es[es <= A_MAX_SRC]
        eb = es[es > A_MAX_SRC]
        if n <= A_MAX_SRC:
            ea = np.append(ea, n)
        else:
            eb = np.append(eb, n)
        ka, kb = len(ea), len(eb)
        oa, ob = fa[c, w], fb[c, w]
        idxA[c, w, oa:oa + ka] = (ea + 1).astype(np.int16)
        dlA[c, w, oa:oa + ka] = p
        idxB[c, w, ob:ob + kb] = (eb - (B_OFF - 1)).astype(np.int16)
        dlB[c, w, ob:ob + kb] = p
        fa[c, w] += ka
        fb[c, w] += kb

    # device layouts
    def wrap_idx(arr, ns):  # [NCORES, WPC, ns] -> [NCORES, 128, WPC*ns//16]
        a = arr.reshape(NCORES, WPC, ns // 16, 16)
        a = np.transpose(a, (0, 3, 1, 2)).reshape(NCORES, 16, WPC * (ns // 16))
        return np.tile(a, (1, 8, 1)).copy()

    def wrap_dl(arr, ns):  # -> [NCORES, 128, WPC*(ns//128)]
        a = arr.reshape(NCORES, WPC, ns // P, P)
        return np.transpose(a, (0, 3, 1, 2)).reshape(NCORES, P, WPC * (ns // P)).copy()

    plan = dict(
        NA=NA, NB=NB, dinv=dinv,
        idxA=wrap_idx(idxA, slots_a), idxB=wrap_idx(idxB, slots_b),
        dlA=wrap_dl(dlA, slots_a), dlB=wrap_dl(dlB, slots_b),
        dinvw=dinvw,                                    # [NCORES, WPC, P]
        dinvp=np.transpose(dinvw, (0, 2, 1)).copy(),    # [NCORES, P, WPC]
        node_core=node_core, node_row=node_win * P + node_pos,
    )
    return plan


def make_table(feat, dinv):
    """[TBL_ROWS, F] f16 table: row n+1 = dinv[n] * feat[n]; rows 0, N+1 zero."""
    t = np.zeros((TBL_ROWS, F), np.float16)
    t[1:N + 1] = (feat * dinv[:, None]).astype(np.float16)
    return t


# ---------------------------------------------------------------- device kernel
def build_kernel(layer, NA, NB, wpc=WPC, use_b=True, s16=False,
                 msg_bufs=2, s_bufs=4, wk_bufs=2, ps_bufs=None, act_drain=False,
                 probe=None):
    """layer 1: table -> z = relu(agg*dinvw @ W1T + b1) @ W2T   (out [6272,128])
    layer 2: table -> out = relu(agg*dinvp + b2)               (out [6272,128])
    """
    f32, f16, i16 = mybir.dt.float32, mybir.dt.float16, mybir.dt.int16
    fdl = f16 if s16 else f32
    if ps_bufs is None:
        ps_bufs = 2 if layer == 1 else 4
    nc = bacc.Bacc("TRN2", debug=False)
    d = {}
    d["table"] = nc.dram_tensor("table", [TBL_ROWS, F], f16, kind="ExternalInput").ap()
    d["idxA"] = nc.dram_tensor("idxA", [P, wpc * NA * 8], i16, kind="ExternalInput").ap()
    d["idxB"] = nc.dram_tensor("idxB", [P, wpc * NB * 8], i16, kind="ExternalInput").ap()
    d["dlA"] = nc.dram_tensor("dlA", [P, wpc * NA], fdl, kind="ExternalInput").ap()
    d["dlB"] = nc.dram_tensor("dlB", [P, wpc * NB], fdl, kind="ExternalInput").ap()
    d["iota"] = nc.dram_tensor("iota", [P, P], fdl, kind="ExternalInput").ap()
    if layer == 1:
        d["dinvw"] = nc.dram_tensor("dinvw", [P, wpc * P], f32, kind="ExternalInput").ap()
        d["w1t"] = nc.dram_tensor("w1t", [P, H], f16, kind="ExternalInput").ap()
        d["b1c"] = nc.dram_tensor("b1c", [P, H // P], f32, kind="ExternalInput").ap()
        d["w2t"] = nc.dram_tensor("w2t", [P, H], f16, kind="ExternalInput").ap()
    else:
        d["dinvp"] = nc.dram_tensor("dinvp", [P, wpc], f32, kind="ExternalInput").ap()
        d["b2r"] = nc.dram_tensor("b2r", [P, P], f32, kind="ExternalInput").ap()
    out_d = nc.dram_tensor("out", [wpc * P, F], f32, kind="ExternalOutput").ap()

    Relu = mybir.ActivationFunctionType.Relu
    Copy = mybir.ActivationFunctionType.Copy

    WPCl = wpc
    with tile.TileContext(nc) as tc:
        with (
            tc.tile_pool(name="cst", bufs=1) as cp,
            tc.tile_pool(name="msg", bufs=msg_bufs) as mp,
            tc.tile_pool(name="s", bufs=s_bufs) as spool,
            tc.tile_pool(name="work", bufs=wk_bufs) as wp,
            tc.tile_pool(name="psum", bufs=ps_bufs, space="PSUM") as pp,
        ):
            def load(name, shape, dtype):
                t = cp.tile(shape, dtype, tag=name)
                nc.sync.dma_start(out=t[:], in_=d[name][:])
                return t

            idxA_t = load("idxA", [P, wpc * NA * 8], i16)
            idxB_t = load("idxB", [P, wpc * NB * 8], i16)
            dlA_t = load("dlA", [P, wpc * NA], fdl)
            dlB_t = load("dlB", [P, wpc * NB], fdl)
            iota_t = load("iota", [P, P], fdl)
            if layer == 1:
                dinvw_t = load("dinvw", [P, wpc * P], f32)
                w1t_t = load("w1t", [P, H], f16)
                b1c_t = load("b1c", [P, H // P], f32)
                w2t_t = load("w2t", [P, H], f16)
            else:
                dinvp_t = load("dinvp", [P, wpc], f32)
                b2r_t = load("b2r", [P, P], f32)

            sconst = None
            if probe == "noS":
                sconst = cp.tile([P, P], f16, tag="sconst")
                nc.vector.tensor_tensor(out=sconst[:],
                                        in0=dlA_t[:, 0:1].to_broadcast([P, P]),
                                        in1=iota_t[:], op=mybir.AluOpType.is_equal)
            for g0 in range(0, wpc, GW):
                nw = min(GW, wpc - g0)
                ja, jb = nw * NA, nw * NB
                msgs16 = {}
                ranges = [("A", NA, dlA_t)] + ([("B", NB, dlB_t)] if use_b else [])
                for rng, nj, idx_t, npc in ([("A", ja, idxA_t, NA)] + ([("B", jb, idxB_t, NB)] if use_b else [])):
                    mt = mp.tile([P, nj * F], f16, tag=f"m{rng}")
                    in_ap = d["table"][:] if rng == "A" else d["table"][B_OFF:TBL_ROWS, :]
                    nc.gpsimd.dma_gather(
                        out_ap=mt[:].rearrange("p (j e) -> p j e", e=F),
                        in_ap=in_ap,
                        idxs_ap=idx_t[:, g0 * npc * 8:(g0 * npc + nj) * 8],
                        num_idxs=(P if probe == "smallG" else nj * P),
                        num_idxs_reg=(P if probe == "smallG" else nj * P),
                        elem_size=F,
                        single_packet=False,
                    )
                    msgs16[rng] = mt

                for wi in range(nw):
                    w = g0 + wi
                    agg = pp.tile([P, P], f32, tag="agg")
                    nmm = NA + (NB if use_b else 0)
                    k = 0
                    for rng, npc, dl_t in ranges:
                        if probe != "noS":
                            sw_t = spool.tile([P, npc * P], f16, tag=f"s{rng}")
                            nc.vector.tensor_tensor(
                                out=sw_t[:].rearrange("p (c e) -> p c e", e=P),
                                in0=dl_t[:, w * npc:(w + 1) * npc]
                                    .unsqueeze(-1).to_broadcast([P, npc, P]),
                                in1=iota_t[:].unsqueeze(1).to_broadcast([P, npc, P]),
                                op=mybir.AluOpType.is_equal,
                            )
                        for c in range(npc):
                            if probe == "noS":
                                s_t = sconst[:]
                            else:
                                s_t = sw_t[:, c * P:(c + 1) * P]
                            mm = msgs16[rng][:, ((wi * npc + c) if probe != "smallG" else 0) * F:((wi * npc + c) if probe != "smallG" else 0) * F + F]
                            if probe == "noPE" and not (k == 0 or k == nmm - 1):
                                k += 1
                                continue
                            if layer == 1:
                                # aggT[f, d] += msg^T @ S
                                nc.tensor.matmul(out=agg[:], lhsT=mm, rhs=s_t,
                                                 start=(k == 0), stop=(k == nmm - 1))
                            else:
                                # agg[d, f] += S^T @ msg
                                nc.tensor.matmul(out=agg[:], lhsT=s_t, rhs=mm,
                                                 start=(k == 0), stop=(k == nmm - 1))
                            k += 1

                    if layer == 1:
                        # aggTs[f, d] = aggT * dinv[dst]  (free-dim broadcast row)
                        aggs = wp.tile([P, P], f16, tag="aggs")
                        dr = dinvw_t[:, w * P:(w + 1) * P]
                        nc.vector.tensor_tensor(out=aggs[:], in0=agg[:], in1=dr,
                                                op=mybir.AluOpType.mult)
                        hts = []
                        hps = pp.tile([P, H], f32, tag="h")
                        for oc in range(H // P):
                            nc.tensor.matmul(
                                out=hps[:, oc * P:(oc + 1) * P],
                                lhsT=w1t_t[:, oc * P:(oc + 1) * P],
                                rhs=aggs[:], start=True, stop=True)
                            ht = wp.tile([P, P], f16, tag=f"ht{oc}")
                            nc.scalar.activation(out=ht[:], in_=hps[:, oc * P:(oc + 1) * P],
                                                 func=Relu, bias=b1c_t[:, oc:oc + 1])
                            hts.append(ht)
                        zps = pp.tile([P, P], f32, tag="z")
                        for ic in range(H // P):
                            nc.tensor.matmul(out=zps[:], lhsT=hts[ic][:],
                                             rhs=w2t_t[:, ic * P:(ic + 1) * P],
                                             start=(ic == 0), stop=(ic == H // P - 1))
                        zsb = wp.tile([P, P], f32, tag="zsb")
                        if act_drain:
                            nc.scalar.activation(out=zsb[:], in_=zps[:], func=Copy)
                        else:
                            nc.vector.tensor_copy(out=zsb[:], in_=zps[:])
                        nc.sync.dma_start(out=out_d[w * P:(w + 1) * P, :], in_=zsb[:])
                    else:
                        u = wp.tile([P, P], f32, tag="u")
                        if act_drain:
                            nc.scalar.activation(out=u[:], in_=agg[:], func=Copy,
                                                 scale=dinvp_t[:, w:w + 1])
                        else:
                            nc.vector.tensor_scalar_mul(u[:], agg[:], dinvp_t[:, w:w + 1])
                        v = wp.tile([P, P], f32, tag="v")
                        nc.vector.tensor_tensor(out=v[:], in0=u[:],
                                                in1=b2r_t[:],
                                                op=mybir.AluOpType.add)
                        y = wp.tile([P, P], f32, tag="y")
                        nc.scalar.activation(out=y[:], in_=v[:], func=Relu)
                        nc.sync.dma_start(out=out_d[w * P:(w + 1) * P, :], in_=y[:])

    nc.compile()
    return nc


# ---------------------------------------------------------------- entry point
def _in_maps(plan, layer, table, W1=None, b1=None, W2=None, b2=None):
    iota = np.broadcast_to(np.arange(P, dtype=np.float32), (P, P)).copy()
    maps = []
    for c in range(NCORES):
        m = dict(table=table, iota=iota,
                 idxA=plan["idxA"][c], idxB=plan["idxB"][c],
                 dlA=plan["dlA"][c], dlB=plan["dlB"][c])
        if layer == 1:
            m["dinvw"] = np.broadcast_to(
                plan["dinvw"][c].reshape(1, WPC * P), (P, WPC * P)).copy()
            m["w1t"] = W1.T.astype(np.float16).copy()
            m["b1c"] = b1.reshape(H // P, P).T.astype(np.float32).copy()
            m["w2t"] = np.concatenate(
                [W2[:, c0 * P:(c0 + 1) * P].T for c0 in range(H // P)], axis=1
            ).astype(np.float16).copy()
        else:
            m["dinvp"] = plan["dinvp"][c]
            m["b2r"] = np.broadcast_to(b2.astype(np.float32), (P, P)).copy()
        maps.append(m)
    return maps


def _gather_nodes(plan, outs):
    """[NCORES][ROWS_PER_CORE, F] core outputs -> [N, F] in node order."""
    allo = np.stack(outs)  # [NCORES, ROWS_PER_CORE, F]
    return allo[plan["node_core"], plan["node_row"]]


def kernel(**inputs):
    x = np.asarray(inputs["x"], np.float32)
    edge_index = np.asarray(inputs["edge_index"])
    W1 = np.asarray(inputs["W1"], np.float32)
    b1 = np.asarray(inputs["b1"], np.float32)
    W2 = np.asarray(inputs["W2"], np.float32)
    b2 = np.asarray(inputs["b2"], np.float32)

    plan = build_plan(edge_index)
    nc1 = build_kernel(1, plan["NA"], plan["NB"], act_drain=True, wk_bufs=3)
    nc2 = build_kernel(2, plan["NA"], plan["NB"], act_drain=True, wk_bufs=3)

    t1 = make_table(x, plan["dinv"])
    r1 = run_bass_kernel_spmd(
        nc1, _in_maps(plan, 1, t1, W1=W1, b1=b1, W2=W2), core_ids=list(range(NCORES)))
    z = _gather_nodes(plan, [r1.results[c]["out"] for c in range(NCORES)])

    t2 = make_table(z, plan["dinv"])
    r2 = run_bass_kernel_spmd(
        nc2, _in_maps(plan, 2, t2, b2=b2), core_ids=list(range(NCORES)))
    y = _gather_nodes(plan, [r2.results[c]["out"] for c in range(NCORES)])

    last_run_info["exec_time_ns"] = [r1.exec_time_ns, r2.exec_time_ns]
    last_run_info["ncs"] = (nc1, nc2)
    return y.astype(np.float32)



# revision 29
# speedup vs baseline: 1.1524x; 1.1524x over previous
"""Trainium2 Bass kernel for 2-layer GCN (N=50000, E=600000, 128->512->128).

Strategy (8 NeuronCores, graph/data parallel over destination nodes):
  - Host: symmetric-normalization is separable (norm = dinv[src]*dinv[dst]);
    gather-table rows are pre-scaled by dinv[src]; the dst-side dinv[dst] is
    applied ON DEVICE as a per-partition activation scale at the final drain
    of each layer (relu commutes with the positive dinv scale; the biases are
    folded in pre-relu via rank-1 "ghost" matmuls of b (x) 1/dinv so the
    deferred scaling stays exact).
  - Nodes are packed into 8*49 windows of <=128 destination nodes, balancing
    per-window edge counts so one SPMD program (fixed shapes) serves all
    cores. Edge slots are split into two source ranges (A: table rows
    [0,32768), B: rows [17234,50002)) so gather indices fit int16, padded per
    window to NA*128 / NB*128 slots.
  - Device, per gather group (schedule of window-group sizes, small first and
    last groups to shorten ramp/tail): dma_gather fp16 source rows -> build
    the group's one-hot S matrices with one wide DVE is_equal per range ->
    PE matmuls accumulate aggregation in PSUM. Layer 1 continues on-chip:
    agg -> @W1^T (+ b1*invd ghost) -> relu -> @W2^T -> *dinv[dst] -> z.
    Layer 2: agg (+ b2*invd ghost) -> relu(dinv[dst]*...) -> out.
  - Host between launches: reshuffles z shards into the layer-2 gather table
    (scaled by dinv), then un-permutes the final output.
"""

import heapq
import numpy as np

import concourse.bacc as bacc
import concourse.mybir as mybir
import concourse.tile as tile
from concourse.bass_utils import run_bass_kernel_spmd

# problem constants (hardcoded per contract)
N = 50000
E = 600000
F = 128          # in/out feature dim
H = 512          # hidden dim
P = 128
NCORES = 8
WPC = 49                  # windows per core
BINS = NCORES * WPC       # 392
ROWS_PER_CORE = WPC * P   # 6272 output rows per core (>= 6250 real)
TBL_ROWS = N + 2          # zero row at 0 and N+1
A_MAX_SRC = 31270         # srcs <= this go to range A (idx = src+1 <= 32767)
B_OFF = 17234             # range B table view starts at this row
B_PAD_IDX = 32767         # row N+1 (zero) relative to B view
SENTINEL = 300.0          # dstloc value that never matches iota 0..127

# gather-group schedule: sizes of consecutive window groups (sum == WPC).
# Pairs keep the pipeline granularity fine (matches PSUM/SBUF buffering) and
# enable paired 512B-descriptor output stores; the final single window
# shortens the tail.
SCHEDULE = [2] * 24 + [1]
assert sum(SCHEDULE) == WPC

last_run_info = {}


# ---------------------------------------------------------------- host planner
def _pack_bins(a_tot, b_tot, cap_a, cap_b):
    """Greedy balanced packing of nodes into BINS bins (<=P nodes, slot caps).
    Returns per-node bin id, or None if packing failed."""
    order = np.argsort(-(a_tot * 3 + b_tot))  # heaviest first
    bin_of = np.full(N, -1, np.int32)
    heap = [(0, 0, 0, b) for b in range(BINS)]  # (aload, bload, count, bin)
    heapq.heapify(heap)
    for n in order:
        a, b = int(a_tot[n]), int(b_tot[n])
        tried = []
        placed = False
        while heap:
            al, bl, cnt, bid = heapq.heappop(heap)
            if cnt >= P:
                continue  # bin full: drop permanently
            if al + a <= cap_a and bl + b <= cap_b:
                bin_of[n] = bid
                heapq.heappush(heap, (al + a, bl + b, cnt + 1, bid))
                placed = True
                break
            tried.append((al, bl, cnt, bid))
            if len(tried) > 256:
                break
        for t in tried:
            heapq.heappush(heap, t)
        if not placed:
            return None
    return bin_of


def build_plan(edge_index):
    src = np.asarray(edge_index[0], dtype=np.int64).astype(np.int32)
    dst = np.asarray(edge_index[1], dtype=np.int64).astype(np.int32)

    deg = np.bincount(dst, minlength=N).astype(np.int64) + 1  # + self loop
    dinv = (1.0 / np.sqrt(deg)).astype(np.float32)

    # self loops are handled densely on-device (identity matmul over a
    # contiguously-loaded per-window block), so only real edges need slots
    is_a = src <= A_MAX_SRC
    a_tot = np.bincount(dst[is_a], minlength=N)
    b_tot = np.bincount(dst[~is_a], minlength=N)

    for na, nb in ((8, 5), (9, 5), (9, 6), (10, 6), (10, 8), (12, 10)):
        bin_of = _pack_bins(a_tot, b_tot, na * P, nb * P)
        if bin_of is not None:
            NA, NB = na, nb
            break
    else:
        raise RuntimeError("bin packing failed")

    # per-bin node lists / positions
    node_core = bin_of // WPC
    node_win = bin_of % WPC
    node_pos = np.zeros(N, np.int32)
    fill = np.zeros(BINS, np.int32)
    for n in range(N):
        b = bin_of[n]
        node_pos[n] = fill[b]
        fill[b] += 1

    # CSR of incoming edges per node (edges only; self loop added below)
    order = np.argsort(dst, kind="stable")
    src_sorted = src[order]
    starts = np.zeros(N + 1, np.int64)
    np.cumsum(np.bincount(dst, minlength=N), out=starts[1:])

    slots_a = NA * P
    slots_b = NB * P
    idxA = np.zeros((NCORES, WPC, slots_a), np.int16)
    idxB = np.full((NCORES, WPC, slots_b), B_PAD_IDX, np.int16)
    dlA = np.full((NCORES, WPC, slots_a), SENTINEL, np.float32)
    dlB = np.full((NCORES, WPC, slots_b), SENTINEL, np.float32)
    fa = np.zeros((NCORES, WPC), np.int32)
    fb = np.zeros((NCORES, WPC), np.int32)
    dinvw = np.zeros((NCORES, WPC, P), np.float32)

    for n in range(N):
        c, w, p = node_core[n], node_win[n], node_pos[n]
        dinvw[c, w, p] = dinv[n]
        es = src_sorted[starts[n]:starts[n + 1]]
        ea = es[es <= A_MAX_SRC]
        eb = es[es > A_MAX_SRC]
        ka, kb = len(ea), len(eb)
        oa, ob = fa[c, w], fb[c, w]
        idxA[c, w, oa:oa + ka] = (ea + 1).astype(np.int16)
        dlA[c, w, oa:oa + ka] = p
        idxB[c, w, ob:ob + kb] = (eb - (B_OFF - 1)).astype(np.int16)
        dlB[c, w, ob:ob + kb] = p
        fa[c, w] += ka
        fb[c, w] += kb

    # device layouts
    def wrap_idx(arr, ns):  # [NCORES, WPC, ns] -> [NCORES, 128, WPC*ns//16]
        a = arr.reshape(NCORES, WPC, ns // 16, 16)
        a = np.transpose(a, (0, 3, 1, 2)).reshape(NCORES, 16, WPC * (ns // 16))
        return np.tile(a, (1, 8, 1)).copy()

    def wrap_dl(arr, ns):  # -> [NCORES, 128, WPC*(ns//128)]
        a = arr.reshape(NCORES, WPC, ns // P, P)
        return np.transpose(a, (0, 3, 1, 2)).reshape(NCORES, P, WPC * (ns // P)).copy()

    # 1/dinv per (core, window-major row): [NCORES, 1, WPC*P]
    invd = np.zeros((NCORES, WPC, P), np.float32)
    nz = dinvw > 0
    invd[nz] = 1.0 / dinvw[nz]

    # node id at (core, position, window), -1 where the slot is empty
    rows_map = np.full((NCORES, P, WPC), -1, np.int64)
    rows_map[node_core, node_pos, node_win] = np.arange(N)

    plan = dict(
        NA=NA, NB=NB, dinv=dinv,
        idxA=wrap_idx(idxA, slots_a), idxB=wrap_idx(idxB, slots_b),
        dlA=wrap_dl(dlA, slots_a).astype(np.float16),
        dlB=wrap_dl(dlB, slots_b).astype(np.float16),
        dinvw=dinvw,                                    # [NCORES, WPC, P]
        dinvp=np.transpose(dinvw, (0, 2, 1)).copy(),    # [NCORES, P, WPC]
        invd=invd.reshape(NCORES, 1, WPC * P),          # [NCORES, 1, WPC*P]
        rows_map=rows_map,                              # [NCORES, P, WPC]
        node_core=node_core, node_row=node_win * P + node_pos,
    )
    return plan


def make_selft(table, plan):
    """Per-core self-loop message blocks: [NCORES, P, WPC*F] f16 where
    [p, w*F:(w+1)*F] = table row of the node at (core, window w, position p)
    (zeros for empty positions via table row 0)."""
    sel = table[plan["rows_map"] + 1]          # [NCORES, P, WPC, F]
    return np.ascontiguousarray(sel.reshape(NCORES, P, WPC * F))


def make_table(feat, dinv):
    """[TBL_ROWS, F] f16 table: row n+1 = dinv[n] * feat[n]; rows 0, N+1 zero."""
    t = np.zeros((TBL_ROWS, F), np.float16)
    t[1:N + 1] = (feat * dinv[:, None]).astype(np.float16)
    return t


# ---------------------------------------------------------------- device kernel
def build_kernel(layer, NA, NB, wpc=WPC, schedule=None, use_b1=False,
                 use_b2=False, msg_bufs=2, s_bufs=6, wk_bufs=4, ps_bufs=None):
    """layer 1: table -> z = dinvdst * (relu(agg @ W1T + b1*invd) @ W2T)
    layer 2: table -> out = relu(dinvdst * agg + b2)        (both [6272,128])
    """
    f32, f16, i16 = mybir.dt.float32, mybir.dt.float16, mybir.dt.int16
    if schedule is None:
        schedule = SCHEDULE if wpc == WPC else [wpc]
    assert sum(schedule) == wpc
    if ps_bufs is None:
        ps_bufs = 2 if layer == 1 else 4
    nc = bacc.Bacc("TRN2", debug=False)
    # f16 constant blob layout (one DMA): dlA, dlB, iota, ident, dinvp[, w1t, w2t]
    oDlA = 0
    oDlB = oDlA + wpc * NA
    oIota = oDlB + wpc * NB
    oIdent = oIota + P
    oW1 = oIdent + P
    CW = oW1 + (2 * H if layer == 1 else 0)
    d = {}
    d["table"] = nc.dram_tensor("table", [TBL_ROWS, F], f16, kind="ExternalInput").ap()
    d["idx"] = nc.dram_tensor("idx", [P, wpc * (NA + NB) * 8], i16, kind="ExternalInput").ap()
    d["cst"] = nc.dram_tensor("cst", [P, CW], f16, kind="ExternalInput").ap()
    d["selft"] = nc.dram_tensor("selft", [P, wpc * F], f16, kind="ExternalInput").ap()
    d["invd"] = nc.dram_tensor("invd", [1, wpc * P], f16, kind="ExternalInput").ap()
    d["dinvp"] = nc.dram_tensor("dinvp", [P, wpc], f32, kind="ExternalInput").ap()
    if layer == 1 and use_b1:
        d["b1row"] = nc.dram_tensor("b1row", [1, H], f16, kind="ExternalInput").ap()
    elif layer == 2 and use_b2:
        d["b2row"] = nc.dram_tensor("b2row", [1, P], f16, kind="ExternalInput").ap()
    # f16 output, pair-interleaved rows: for window pair k = (2k, 2k+1),
    # physical row k*256 + 2*d + j holds (window 2k+j, position d); the odd
    # final window stays row-major at the end. Pairing makes each store
    # descriptor 512B (full-rate DMA).
    out_d = nc.dram_tensor("out", [wpc * P, F], f16, kind="ExternalOutput").ap()

    Relu = mybir.ActivationFunctionType.Relu
    Copy = mybir.ActivationFunctionType.Copy

    # group start offsets
    starts = []
    g0 = 0
    for nw in schedule:
        starts.append(g0)
        g0 += nw

    with tile.TileContext(nc) as tc:
        with (
            tc.tile_pool(name="cst", bufs=1) as cp,
            tc.tile_pool(name="msg", bufs=msg_bufs) as mp,
            tc.tile_pool(name="selfp", bufs=4) as sfp,
            tc.tile_pool(name="s", bufs=s_bufs) as spool,
            tc.tile_pool(name="work", bufs=wk_bufs) as wp,
            tc.tile_pool(name="psum", bufs=ps_bufs, space="PSUM") as pp,
            tc.tile_pool(name="psum_h", bufs=3, space="PSUM") as pph,
            tc.tile_pool(name="psum_z", bufs=3, space="PSUM") as ppz,
        ):
            def load(name, shape, dtype):
                t = cp.tile(shape, dtype, tag=name)
                nc.sync.dma_start(out=t[:], in_=d[name][:])
                return t

            # first-group index slices load first (tiny) so gathers start early
            nw0 = schedule[0]
            oIB = wpc * NA * 8
            idxA0 = cp.tile([P, nw0 * NA * 8], i16, tag="idxA0")
            nc.sync.dma_start(out=idxA0[:], in_=d["idx"][:, :nw0 * NA * 8])
            idxB0 = cp.tile([P, nw0 * NB * 8], i16, tag="idxB0")
            nc.sync.dma_start(out=idxB0[:], in_=d["idx"][:, oIB:oIB + nw0 * NB * 8])
            cst_t = load("cst", [P, CW], f16)
            idx_t = load("idx", [P, wpc * (NA + NB) * 8], i16)
            invd_t = load("invd", [1, wpc * P], f16)
            dinvp_t = load("dinvp", [P, wpc], f32)
            if layer == 1 and use_b1:
                b1row_t = load("b1row", [1, H], f16)
            elif layer == 2 and use_b2:
                b2row_t = load("b2row", [1, P], f16)

            for gi, (g0, nw) in enumerate(zip(starts, schedule)):
                ja, jb = nw * NA, nw * NB
                # group's self-loop message block (contiguous rows, cheap DMA)
                selfw = sfp.tile([P, nw * F], f16, tag="selfw")
                nc.sync.dma_start(out=selfw[:],
                                  in_=d["selft"][:, g0 * F:(g0 + nw) * F])
                msgs16 = {}
                for rng, nj, it, npc, rb in (
                    ("A", ja, (idxA0 if gi == 0 else idx_t), NA, 0),
                    ("B", jb, (idxB0 if gi == 0 else idx_t), NB, wpc * NA * 8),
                ):
                    mt = mp.tile([P, nj * F], f16, tag=f"m{rng}")
                    in_ap = d["table"][:] if rng == "A" else d["table"][B_OFF:TBL_ROWS, :]
                    off = 0 if gi == 0 else rb + g0 * npc * 8
                    nc.gpsimd.dma_gather(
                        out_ap=mt[:].rearrange("p (j e) -> p j e", e=F),
                        in_ap=in_ap,
                        idxs_ap=it[:, off:off + nj * 8],
                        num_idxs=nj * P,
                        num_idxs_reg=nj * P,
                        elem_size=F,
                        single_packet=False,
                    )
                    msgs16[rng] = mt

                aggs_of = {}
                for wi in range(nw):
                    w = g0 + wi
                    # per-window S builds (fine granularity keeps PE fed)
                    sw = {}
                    for rng, npc, odl in (("A", NA, oDlA), ("B", NB, oDlB)):
                        st = spool.tile([P, npc * P], f16, tag=f"s{rng}")
                        nc.vector.tensor_tensor(
                            out=st[:].rearrange("p (c e) -> p c e", e=P),
                            in0=cst_t[:, odl + w * npc:odl + (w + 1) * npc]
                                .unsqueeze(-1).to_broadcast([P, npc, P]),
                            in1=cst_t[:, oIota:oIdent]
                                .unsqueeze(1).to_broadcast([P, npc, P]),
                            op=mybir.AluOpType.is_equal,
                        )
                        sw[rng] = st
                    agg = pp.tile([P, P], f32, tag="agg")
                    aggs_of[wi] = agg
                    sl = selfw[:, wi * F:(wi + 1) * F]
                    k = 0
                    for rng, npc in (("A", NA), ("B", NB)):
                        for c in range(npc):
                            s_t = sw[rng][:, c * P:(c + 1) * P]
                            mm = msgs16[rng][:, (wi * npc + c) * F:(wi * npc + c) * F + F]
                            if layer == 1:
                                # aggT[f, d] += msg^T @ S
                                nc.tensor.matmul(out=agg[:], lhsT=mm, rhs=s_t,
                                                 start=(k == 0), stop=False,
                                                 skip_group_check=True)
                            else:
                                # agg[d, f] += S^T @ msg
                                nc.tensor.matmul(out=agg[:], lhsT=s_t, rhs=mm,
                                                 start=(k == 0), stop=False,
                                                 skip_group_check=True)
                            k += 1
                    # dense self-loop block, accumulated last
                    if layer == 1:
                        # aggT[f, d] += self[d, f]^T
                        nc.tensor.matmul(out=agg[:], lhsT=sl, rhs=cst_t[:, oIdent:oW1],
                                         start=False, stop=True,
                                         skip_group_check=True)
                    else:
                        # agg[d, f] += self[d, f]
                        nc.tensor.matmul(out=agg[:], lhsT=cst_t[:, oIdent:oW1], rhs=sl,
                                         start=False, stop=not use_b2,
                                         skip_group_check=True)
                        if use_b2:
                            # ghost: agg[d, f] += invd[d] * b2[f] (pre-relu bias)
                            nc.tensor.matmul(out=agg[:],
                                             lhsT=invd_t[:, w * P:(w + 1) * P],
                                             rhs=b2row_t[:],
                                             start=False, stop=True,
                                             skip_group_check=True)

                # second pass: transform/output stages (agg chains of the whole
                # group are already queued, so PE never waits on ACT here)
                for wi in range(nw):
                    w = g0 + wi
                    agg = aggs_of[wi]
                    if layer == 1:
                        aggs = wp.tile([P, P], f16, tag="aggs")
                        nc.scalar.activation(out=aggs[:], in_=agg[:], func=Copy)
                        hts = wp.tile([P, H], f16, tag="hts")
                        if use_b1:
                            hps = pph.tile([P, H], f32, tag="h")
                            for oc in range(H // P):
                                nc.tensor.matmul(
                                    out=hps[:, oc * P:(oc + 1) * P],
                                    lhsT=cst_t[:, oW1 + oc * P:oW1 + (oc + 1) * P],
                                    rhs=aggs[:], start=True, stop=False)
                                # ghost: hp[h, d] += b1[h] * invd[d]
                                nc.tensor.matmul(
                                    out=hps[:, oc * P:(oc + 1) * P],
                                    lhsT=b1row_t[:, oc * P:(oc + 1) * P],
                                    rhs=invd_t[:, w * P:(w + 1) * P],
                                    start=False, stop=True, skip_group_check=True)
                            nc.scalar.activation(out=hts[:], in_=hps[:], func=Relu)
                        else:
                            hps = pph.tile([P, H], f32, tag="h")
                            for oc in range(H // P):
                                nc.tensor.matmul(
                                    out=hps[:, oc * P:(oc + 1) * P],
                                    lhsT=cst_t[:, oW1 + oc * P:oW1 + (oc + 1) * P],
                                    rhs=aggs[:], start=True, stop=True)
                            # one wide relu drain for all four chunks
                            nc.scalar.activation(out=hts[:], in_=hps[:], func=Relu)
                        zps = ppz.tile([P, P], f32, tag="z")
                        for ic in range(H // P):
                            nc.tensor.matmul(out=zps[:], lhsT=hts[:, ic * P:(ic + 1) * P],
                                             rhs=cst_t[:, oW1 + H + ic * P:oW1 + H + (ic + 1) * P],
                                             start=(ic == 0), stop=(ic == H // P - 1))
                        paired = nw % 2 == 0
                        if paired and wi % 2 == 0:
                            pair = wp.tile([P, 2 * F], f16, tag="pair")
                        if paired:
                            tgt = pair[:, (wi % 2) * F:(wi % 2 + 1) * F]
                        else:
                            single = wp.tile([P, F], f16, tag="single")
                            tgt = single[:]
                        # deferred dst-side normalization (relu-commuted)
                        nc.scalar.activation(out=tgt, in_=zps[:], func=Copy,
                                             scale=dinvp_t[:, w:w + 1])
                    else:
                        paired = nw % 2 == 0
                        if paired and wi % 2 == 0:
                            pair = wp.tile([P, 2 * F], f16, tag="pair")
                        if paired:
                            tgt = pair[:, (wi % 2) * F:(wi % 2 + 1) * F]
                        else:
                            single = wp.tile([P, F], f16, tag="single")
                            tgt = single[:]
                        nc.scalar.activation(out=tgt, in_=agg[:], func=Relu,
                                             scale=dinvp_t[:, w:w + 1])
                    if paired and wi % 2 == 1:
                        # one 512B-per-descriptor store for the window pair
                        nc.sync.dma_start(
                            out=out_d[(w - 1) * P:(w + 1) * P, :]
                                .rearrange("(p j) f -> p (j f)", j=2),
                            in_=pair[:])
                    elif not paired:
                        nc.sync.dma_start(out=out_d[w * P:(w + 1) * P, :],
                                          in_=single[:])

    nc.compile()
    return nc


# ---------------------------------------------------------------- entry point
def _in_maps(plan, layer, table, W1=None, b1=None, W2=None, b2=None):
    iota = np.broadcast_to(np.arange(P, dtype=np.float16), (P, P))
    ident = np.eye(P, dtype=np.float16)
    selft = make_selft(table, plan)
    if layer == 1:
        w1t = W1.T.astype(np.float16)
        w2t = np.concatenate(
            [W2[:, c0 * P:(c0 + 1) * P].T for c0 in range(H // P)], axis=1
        ).astype(np.float16)
    maps = []
    for c in range(NCORES):
        parts = [plan["dlA"][c], plan["dlB"][c], iota, ident]
        if layer == 1:
            parts += [w1t, w2t]
        cst = np.ascontiguousarray(np.concatenate(parts, axis=1))
        idx = np.ascontiguousarray(
            np.concatenate([plan["idxA"][c], plan["idxB"][c]], axis=1))
        m = dict(table=table, cst=cst, idx=idx, selft=selft[c],
                 dinvp=plan["dinvp"][c],
                 invd=plan["invd"][c].astype(np.float16))
        if layer == 1 and b1 is not None and np.any(b1):
            m["b1row"] = b1.reshape(1, H).astype(np.float16).copy()
        if layer == 2 and b2 is not None and np.any(b2):
            m["b2row"] = b2.reshape(1, P).astype(np.float16).copy()
        maps.append(m)
    return maps


def _phys_perm(schedule=None):
    """logical row (w*P + d) -> physical out row under pair-interleaving."""
    if schedule is None:
        schedule = SCHEDULE
    perm = np.zeros(WPC * P, np.int64)
    ar = np.arange(P)
    g0 = 0
    for nw in schedule:
        if nw == 2:
            for j in range(2):
                perm[(g0 + j) * P + ar] = g0 * P + 2 * ar + j
        else:
            for j in range(nw):
                perm[(g0 + j) * P + ar] = (g0 + j) * P + ar
        g0 += nw
    return perm


def _gather_nodes(plan, outs):
    """[NCORES][ROWS_PER_CORE, F] core outputs -> [N, F] in node order."""
    allo = np.stack(outs)  # [NCORES, ROWS_PER_CORE, F]
    perm = _phys_perm()
    return allo[plan["node_core"], perm[plan["node_row"]]].astype(np.float32)


def kernel(**inputs):
    x = np.asarray(inputs["x"], np.float32)
    edge_index = np.asarray(inputs["edge_index"])
    W1 = np.asarray(inputs["W1"], np.float32)
    b1 = np.asarray(inputs["b1"], np.float32)
    W2 = np.asarray(inputs["W2"], np.float32)
    b2 = np.asarray(inputs["b2"], np.float32)

    plan = build_plan(edge_index)
    nc1 = build_kernel(1, plan["NA"], plan["NB"], use_b1=bool(np.any(b1)),
                       wk_bufs=8, msg_bufs=4, s_bufs=16)
    nc2 = build_kernel(2, plan["NA"], plan["NB"], use_b2=bool(np.any(b2)),
                       wk_bufs=8, msg_bufs=4, s_bufs=16)

    t1 = make_table(x, plan["dinv"])
    r1 = run_bass_kernel_spmd(
        nc1, _in_maps(plan, 1, t1, W1=W1, b1=b1, W2=W2), core_ids=list(range(NCORES)))
    z = _gather_nodes(plan, [r1.results[c]["out"] for c in range(NCORES)])

    t2 = make_table(z, plan["dinv"])
    r2 = run_bass_kernel_spmd(
        nc2, _in_maps(plan, 2, t2, b2=b2), core_ids=list(range(NCORES)))
    y = _gather_nodes(plan, [r2.results[c]["out"] for c in range(NCORES)])

    last_run_info["exec_time_ns"] = [r1.exec_time_ns, r2.exec_time_ns]
    last_run_info["ncs"] = (nc1, nc2)
    return y.astype(np.float32)
